# revision 1
# baseline (speedup 1.0000x reference)
"""KNN-classifier kernel for Trainium2 (8 NeuronCores, SPMD).

Strategy:
  - Shard train_features row-wise across 8 cores (12500 rows each).
  - Per core: sim = features_rank @ shard.T computed on the PE array with a
    3-pass fp16 split (q = qh + ql, t = th + tl; sim = qh*th + qh*tl + ql*th
    accumulated in fp32 PSUM) -> exact-fp32-level accuracy at 1 cycle/row.
  - Per 512-column tile: top-8 values + indices via DVE max/max_index.
  - Host: merge the 8 x 200 candidates/row, take global top-200, softmax,
    weighted class histograms (exactly mirroring the reference math).

The softmax at T=0.07 underflows to exactly 0 (fp32) for any neighbor more
than ~7 below the row max; on this data top1-top9 >= 3.8 for every row, so
per-tile top-8 candidates capture every neighbor with non-negligible weight.
"""

import sys

sys.path.insert(0, "/opt/trn_rl_repo")

import numpy as np

B = 2048
D = 1024
NTRAIN = 100000
NCORES = 8
NLOC = NTRAIN // NCORES    # 12500
TS = 512                   # free-dim tile (one fp32 PSUM bank)
KC = D // 128              # 8 contraction chunks
MAXK = 200
TEMP = 0.07
NB_KNN = (10, 20, 100, 200)
NUM_CLASSES = 1000

_CACHE = {}


def _build(bt, nloc):
    """Emit the SPMD Bass program for `bt*128` query rows x `nloc` train rows."""
    from concourse import bass, tile, mybir

    # The PJRT compile path encodes at most one sync-wait per TPB pseudo
    # instruction; Tile's kernel-tail drain collects one wait per logical
    # processor. Split it into a chain of single-wait drains (same SP queue,
    # executed in order -> semantically identical).
    if not getattr(tile.TileContext, "_drain_split_patched", False):
        from concourse.vector_clock import ScopedClock

        def _split_drain(self, tick_clock, wait_clock):
            drain_inst = self.nc.sync.drain()
            wait_clock.add_sem_waits(
                drain_inst.ins, ScopedClock({None: tick_clock.global_clock})
            )
            si = drain_inst.ins.sync_info
            if si is not None and si.on_wait and len(si.on_wait) > 1:
                waits = list(si.on_wait)
                try:
                    si.on_wait[:] = waits[:1]
                except Exception:
                    drain_inst.ins.sync_info = mybir.SyncInfo(
                        on_wait=waits[:1], on_update=list(si.on_update))
                for wt in waits[1:]:
                    d2 = self.nc.sync.drain()
                    s2 = d2.ins.sync_info
                    if s2 is None:
                        d2.ins.sync_info = mybir.SyncInfo(
                            on_wait=[wt], on_update=[])
                    else:
                        try:
                            s2.on_wait[:] = [wt]
                        except Exception:
                            d2.ins.sync_info = mybir.SyncInfo(
                                on_wait=[wt], on_update=list(s2.on_update))
            self.nc.all_engine_barrier()
            popped = self.nc._tile_sem_poison_stack.pop()
            assert popped is self._sem_poison
            self.nc.clear_and_free_semaphores(
                list(self.sems.allocated().values()))
            self.nc.all_engine_barrier()

        tile.TileContext._drain_and_barrier = _split_drain
        tile.TileContext._drain_split_patched = True

    F16 = mybir.dt.float16
    F32 = mybir.dt.float32
    U32 = mybir.dt.uint32

    nt = (nloc + TS - 1) // TS
    cpt = nt * 8  # candidates per row
    nb = bt * 128

    nc = bass.Bass()
    qT = nc.declare_dram_parameter("qT", [2 * D, nb], F16, isOutput=False)
    tT = nc.declare_dram_parameter("tT", [2 * D, nloc], F16, isOutput=False)
    out_all = nc.declare_dram_parameter("out_all", [nb, 2 * cpt], U32, isOutput=True)

    qT3 = qT.rearrange("(k p) b -> p k b", p=128)   # k: 0..7 hi, 8..15 lo
    tT3 = tT.rearrange("(k p) n -> p k n", p=128)
    out3 = out_all.rearrange("(b p) c -> p b c", p=128)

    with tile.TileContext(nc) as tc:
        with (
            tc.tile_pool(name="qpool", bufs=1) as qpool,
            tc.tile_pool(name="spool", bufs=1) as spool,
            tc.tile_pool(name="ppool", bufs=6, space="PSUM") as ppool,
        ):
            # everything SBUF-resident: 4 input DMAs on SW lanes, 2 output
            # DMAs on HW lanes -> no DGE lane reuse, every DMA <= 1 wait
            # (DIRECT2D descriptors encode at most one sync-wait).
            q16 = qpool.tile([128, 2 * KC, nb], F16)
            t16 = qpool.tile([128, 2 * KC, nloc], F16)
            nc.gpsimd.dma_start(out=q16[:], in_=qT3[:])
            nc.gpsimd.dma_start(out=t16[:], in_=tT3[:])

            all32 = spool.tile([128, bt * 2 * cpt], U32)

            for t in range(nt):
                w = min(TS, nloc - t * TS)
                ns = slice(t * TS, t * TS + w)
                for b in range(bt):
                    ps = ppool.tile([128, w], F32, tag="ps")
                    bs = slice(b * 128, (b + 1) * 128)
                    for k in range(KC):
                        nc.tensor.matmul(
                            out=ps[:], lhsT=q16[:, k, bs], rhs=t16[:, k, ns],
                            start=(k == 0), stop=False,
                        )
                        nc.tensor.matmul(
                            out=ps[:], lhsT=q16[:, k, bs], rhs=t16[:, KC + k, ns],
                            start=False, stop=False,
                        )
                    for k in range(KC):
                        nc.tensor.matmul(
                            out=ps[:], lhsT=q16[:, KC + k, bs], rhs=t16[:, k, ns],
                            start=False, stop=(k == KC - 1),
                        )
                    vsl = slice(b * 2 * cpt + t * 8, b * 2 * cpt + t * 8 + 8)
                    isl = slice(b * 2 * cpt + cpt + t * 8, b * 2 * cpt + cpt + t * 8 + 8)
                    nc.vector.max(out=all32[:, vsl].bitcast(F32), in_=ps[:])
                    nc.vector.max_index(
                        out=all32[:, isl], in_max=all32[:, vsl].bitcast(F32),
                        in_values=ps[:],
                    )
            nc.gpsimd.dma_start(out=out3[:], in_=all32[:])
    return nc


def _split16(x):
    hi = x.astype(np.float16)
    lo = (x - hi.astype(np.float32)).astype(np.float16)
    return hi, lo


ROUNDS = 4  # sequential launches; each holds its train shard fully in SBUF


def _run_device(q, t, trace=False):
    """Returns (vals [B,8*cpt] f32, gidx [B,8*cpt] int64) candidate arrays."""
    from concourse.bass_utils import run_bass_kernel_spmd

    bt = q.shape[0] // 128
    nloc = t.shape[0] // NCORES
    nt = (nloc + TS - 1) // TS
    cpt = nt * 8

    key = (bt, nloc)
    if key not in _CACHE:
        _CACHE[key] = _build(bt, nloc)
    nc = _CACHE[key]

    qh, ql = _split16(q)
    qT = np.ascontiguousarray(np.concatenate([qh.T, ql.T], axis=0))
    in_maps = []
    for c in range(NCORES):
        th, tl = _split16(t[c * nloc:(c + 1) * nloc])
        in_maps.append({
            "qT": qT,
            "tT": np.ascontiguousarray(np.concatenate([th.T, tl.T], axis=0)),
        })
    res = run_bass_kernel_spmd(nc, in_maps, core_ids=list(range(NCORES)), trace=trace)
    if trace:
        _run_device.last_exec_ns = res.exec_time_ns

    outs = [res.results[c]["out_all"].reshape(-1, 2, cpt) for c in range(NCORES)]
    vals = np.stack([o[:, 0, :].view(np.float32) for o in outs])  # [8,B,cpt]
    idxs = np.stack([o[:, 1, :] for o in outs])
    tile_base = np.arange(nt, dtype=np.int64).repeat(8) * TS              # [cpt]
    base = np.arange(NCORES, dtype=np.int64)[:, None] * nloc + tile_base[None, :]
    gidx = idxs.astype(np.int64) + base[:, None, :]
    bsz = q.shape[0]
    cv = vals.transpose(1, 0, 2).reshape(bsz, NCORES * cpt)
    ci = gidx.transpose(1, 0, 2).reshape(bsz, NCORES * cpt)
    return cv, ci


def kernel(features_rank, train_features, train_labels):
    q = np.ascontiguousarray(np.asarray(features_rank), dtype=np.float32)
    t = np.ascontiguousarray(np.asarray(train_features), dtype=np.float32)
    lab = np.asarray(train_labels)

    nlr = NLOC // ROUNDS
    cvs, cis = [], []
    for r in range(ROUNDS):
        tr = np.ascontiguousarray(np.concatenate(
            [t[c * NLOC + r * nlr:c * NLOC + (r + 1) * nlr] for c in range(NCORES)],
            axis=0))
        cv_r, ci_r = _run_device(q, tr)
        c_id, local = ci_r // nlr, ci_r % nlr
        cvs.append(cv_r)
        cis.append(c_id * NLOC + r * nlr + local)
    cv = np.concatenate(cvs, axis=1)
    ci = np.concatenate(cis, axis=1)

    # global top-MAXK, sorted desc by value then asc by index (jax tie order)
    order = np.lexsort((ci, -cv), axis=1)[:, :MAXK]
    topv = np.take_along_axis(cv, order, axis=1).astype(np.float32)
    topi = np.take_along_axis(ci, order, axis=1)
    nl = lab[topi]

    x = (topv / np.float32(TEMP)).astype(np.float32)
    x -= x.max(axis=1, keepdims=True)
    e = np.exp(x, dtype=np.float32)
    wts = (e / e.sum(axis=1, keepdims=True, dtype=np.float32)).astype(np.float32)

    bsz = q.shape[0]
    rows = np.arange(bsz)[:, None]
    probas = []
    for k in NB_KNN:
        p = np.zeros((bsz, NUM_CLASSES), np.float32)
        np.add.at(p, (np.broadcast_to(rows, (bsz, k)), nl[:, :k]), wts[:, :k])
        probas.append(p)
    return tuple(probas)



# revision 5
# speedup vs baseline: 31430.3573x; 31430.3573x over previous
"""KNN-classifier kernel for Trainium2 (8 NeuronCores, SPMD).

Strategy:
  - Shard train_features row-wise across 8 cores (12500 rows each).
  - Per core, single launch: sim = features_rank @ shard.T on the PE array
    in ONE fp16 pass (inputs rounded to fp16, fp32 PSUM accumulation).
    t-shard streamed through SBUF in 5 double-buffered chunks; q resident.
  - Per 500-column tile: top-8 values + indices via DVE max/max_index
    (k-outer/tile-inner matmul order reuses each loaded weight 5x).
  - Host: merge the 8 x 200 candidates/row, take global top-200 by the
    fp16-accurate sims, then exactly rescore (fp32 dot) the top-64 of the
    few rows whose 2nd softmax weight is non-negligible -- at T=0.07 the
    softmax is so sharp that every other row's output is determined by
    its top-1 neighbor to well below the tolerance. Then softmax +
    weighted class histograms exactly mirroring the reference math.

The fp16 rounding error on a 1024-dim dot of randn vectors is ~6e-3 std
(max ~0.05 over the 205M sims); per-512-tile top-8 selection and global
top-200 membership have inter-candidate gaps orders of magnitude larger,
so only rows with top-2 sim gap < ~0.6 (about 150/2048) need the exact
rescore. Validated end-to-end in numpy: rel_err 5.3e-4 (gate 2e-2).
"""

import sys

sys.path.insert(0, "/opt/trn_rl_repo")

import numpy as np

B = 2048
D = 1024
NTRAIN = 100000
NCORES = 8
NLOC = NTRAIN // NCORES    # 12500
TS = 500                   # free-dim tile (<= one fp32 PSUM bank)
NT = NLOC // TS            # 25 tiles
GT = 5                     # tiles per streamed t-chunk
NG = NT // GT              # 5 chunks
KC = D // 128              # 8 contraction chunks
BT = B // 128              # 16 query blocks
CPT = NT * 8               # 200 candidates per row per core
MAXK = 200
TEMP = 0.07
NB_KNN = (10, 20, 100, 200)
NUM_CLASSES = 1000
RESCORE_W2 = 1e-4          # rescore rows whose 2nd approx weight exceeds this
RESCORE_POOL = 64          # exact-rescore pool per ambiguous row

_CACHE = {}


def _build():
    """Emit the single-launch SPMD Bass program (one 12500-row shard/core)."""
    from concourse import bass, tile, mybir

    # The PJRT compile path encodes at most one sync-wait per TPB pseudo
    # instruction; Tile's kernel-tail drain collects one wait per logical
    # processor. Split it into a chain of single-wait drains (same SP queue,
    # executed in order -> semantically identical).
    if not getattr(tile.TileContext, "_drain_split_patched", False):
        from concourse.vector_clock import ScopedClock

        def _split_drain(self, tick_clock, wait_clock):
            drain_inst = self.nc.sync.drain()
            wait_clock.add_sem_waits(
                drain_inst.ins, ScopedClock({None: tick_clock.global_clock})
            )
            si = drain_inst.ins.sync_info
            if si is not None and si.on_wait and len(si.on_wait) > 1:
                waits = list(si.on_wait)
                try:
                    si.on_wait[:] = waits[:1]
                except Exception:
                    drain_inst.ins.sync_info = mybir.SyncInfo(
                        on_wait=waits[:1], on_update=list(si.on_update))
                for wt in waits[1:]:
                    d2 = self.nc.sync.drain()
                    s2 = d2.ins.sync_info
                    if s2 is None:
                        d2.ins.sync_info = mybir.SyncInfo(
                            on_wait=[wt], on_update=[])
                    else:
                        try:
                            s2.on_wait[:] = [wt]
                        except Exception:
                            d2.ins.sync_info = mybir.SyncInfo(
                                on_wait=[wt], on_update=list(s2.on_update))
            self.nc.all_engine_barrier()
            popped = self.nc._tile_sem_poison_stack.pop()
            assert popped is self._sem_poison
            self.nc.clear_and_free_semaphores(
                list(self.sems.allocated().values()))
            self.nc.all_engine_barrier()

        tile.TileContext._drain_and_barrier = _split_drain
        tile.TileContext._drain_split_patched = True

    F16 = mybir.dt.float16
    F32 = mybir.dt.float32
    U32 = mybir.dt.uint32

    nc = bass.Bass()
    qT = nc.declare_dram_parameter("qT", [D, B], F16, isOutput=False)
    tT = nc.declare_dram_parameter("tT", [D, NLOC], F16, isOutput=False)
    out_all = nc.declare_dram_parameter("out_all", [B, 2 * CPT], U32, isOutput=True)

    qT3 = qT.rearrange("(k p) b -> p k b", p=128)
    tT3 = tT.rearrange("(k p) n -> p k n", p=128)
    out3 = out_all.rearrange("(b p) c -> p b c", p=128)

    with tile.TileContext(nc) as tc:
        with (
            tc.tile_pool(name="qpool", bufs=1) as qpool,
            tc.tile_pool(name="tpool", bufs=2) as tpool,
            tc.tile_pool(name="spool", bufs=1) as spool,
            tc.tile_pool(name="ppool", bufs=8, space="PSUM") as ppool,
        ):
            q16 = qpool.tile([128, KC, B], F16)
            nc.gpsimd.dma_start(out=q16[:], in_=qT3[:])
            all32 = spool.tile([128, BT * 2 * CPT], U32)

            for g in range(NG):
                t16 = tpool.tile([128, KC, GT * TS], F16, tag="t16")
                nc.gpsimd.dma_start(
                    out=t16[:], in_=tT3[:, :, g * GT * TS:(g + 1) * GT * TS])
                # Wait-absorbers: the PJRT compile path allows at most ONE
                # sync-wait per TPB instruction, but the first matmul of a
                # chunk would need to wait on both the t16 DMA and a PSUM
                # slot release. A standalone ldweights (PE-queue, reads the
                # fresh tile, writes nothing) takes the DMA wait instead.
                if g == 0:
                    nc.tensor.ldweights(weights=q16[:, 0, 0:128])
                nc.tensor.ldweights(weights=t16[:, 0, 0:128])
                for b in range(BT):
                    bs = slice(b * 128, (b + 1) * 128)
                    pss = [ppool.tile([128, TS], F32, tag="ps",
                                      name=f"ps_{g}_{b}_{i}")
                           for i in range(GT)]
                    # k-outer / tile-inner: each q weight load feeds GT
                    # matmuls back-to-back.
                    for k in range(KC):
                        for ti in range(GT):
                            nc.tensor.matmul(
                                out=pss[ti][:], lhsT=q16[:, k, bs],
                                rhs=t16[:, k, ti * TS:(ti + 1) * TS],
                                start=(k == 0), stop=(k == KC - 1),
                            )
                    for ti in range(GT):
                        tg = g * GT + ti
                        base = b * 2 * CPT
                        vsl = slice(base + tg * 8, base + tg * 8 + 8)
                        isl = slice(base + CPT + tg * 8, base + CPT + tg * 8 + 8)
                        nc.vector.max(
                            out=all32[:, vsl].bitcast(F32), in_=pss[ti][:])
                        nc.vector.max_index(
                            out=all32[:, isl],
                            in_max=all32[:, vsl].bitcast(F32),
                            in_values=pss[ti][:],
                        )
            nc.gpsimd.dma_start(out=out3[:], in_=all32[:])

    # The PJRT DMA descriptors encode at most one sync-wait. The steady-state
    # t16 chunk DMAs get two: PE >= (slot readers done) and DMASW >= (the
    # previous DMA into this slot done, WAW). The WAW wait is transitively
    # implied by the PE wait -- every PE reader of the slot already waited on
    # that DMA's completion sem -- so drop it, keeping the PE wait.
    for blk in nc.m.functions[0].blocks:
        for ins in blk.instructions:
            si = getattr(ins, "sync_info", None)
            if si is None or not si.on_wait or len(si.on_wait) <= 1:
                continue
            assert type(ins).__name__ == "InstDMACopy", ins
            pe = [w for w in si.on_wait if w.ant_name.startswith("PE")]
            dmasw = [w for w in si.on_wait if w.ant_name.startswith("DMASW")]
            assert len(pe) == 1 and len(pe) + len(dmasw) == len(si.on_wait), (
                si.on_wait)
            try:
                si.on_wait[:] = pe
            except Exception:
                ins.sync_info = mybir.SyncInfo(
                    on_wait=pe, on_update=list(si.on_update))
    return nc


def _run_device(q, t, trace=False):
    """Returns (vals [B, 8*CPT] f32, gidx [B, 8*CPT] int64) candidates."""
    from concourse.bass_utils import run_bass_kernel_spmd

    if "nc" not in _CACHE:
        _CACHE["nc"] = _build()
    nc = _CACHE["nc"]

    q16 = np.ascontiguousarray(q.astype(np.float16).T)          # [D, B]
    in_maps = []
    for c in range(NCORES):
        sh = t[c * NLOC:(c + 1) * NLOC].astype(np.float16)
        in_maps.append({
            "qT": q16,
            "tT": np.ascontiguousarray(sh.T),                    # [D, NLOC]
        })
    res = run_bass_kernel_spmd(nc, in_maps, core_ids=list(range(NCORES)),
                               trace=trace)
    if trace:
        _run_device.last_exec_ns = res.exec_time_ns

    tile_base = np.arange(NT, dtype=np.int64).repeat(8) * TS     # [CPT]
    cvs, cis = [], []
    for c in range(NCORES):
        o = res.results[c]["out_all"]                            # [B, 2*CPT] u32
        cvs.append(o.view(np.float32)[:, :CPT])
        cis.append(o[:, CPT:].astype(np.int64) + (c * NLOC + tile_base))
    return np.concatenate(cvs, axis=1), np.concatenate(cis, axis=1)


def kernel(features_rank, train_features, train_labels):
    q = np.ascontiguousarray(np.asarray(features_rank), dtype=np.float32)
    t = np.ascontiguousarray(np.asarray(train_features), dtype=np.float32)
    lab = np.asarray(train_labels)

    cv, ci = _run_device(q, t)                                   # [B, 1600]

    # global top-MAXK by approx value (desc, ties by index asc = jax order)
    part = np.argpartition(-cv, MAXK - 1, axis=1)[:, :MAXK]
    pv = np.take_along_axis(cv, part, axis=1)
    pi = np.take_along_axis(ci, part, axis=1)
    order = np.lexsort((pi, -pv), axis=1)
    topv = np.take_along_axis(pv, order, axis=1)
    topi = np.take_along_axis(pi, order, axis=1)

    # approx softmax weights to find rows whose output is sensitive to the
    # fp16 sim noise (non-negligible 2nd weight)
    x = topv / np.float32(TEMP)
    x -= x[:, :1]                                   # topv sorted desc
    e = np.exp(x, dtype=np.float32)
    w = e / e.sum(axis=1, keepdims=True, dtype=np.float32)
    amb = np.where(w[:, 1] > RESCORE_W2)[0]
    if amb.size:
        p2 = RESCORE_POOL
        sub_i = topi[amb, :p2]
        ex = np.einsum("akd,ad->ak", t[sub_i], q[amb],
                       dtype=np.float32, casting="same_kind")
        o2 = np.lexsort((sub_i, -ex), axis=1)
        topv[amb, :p2] = np.take_along_axis(ex, o2, axis=1)
        topi[amb, :p2] = np.take_along_axis(sub_i, o2, axis=1)
        x = topv / np.float32(TEMP)
        x -= x.max(axis=1, keepdims=True)
        e = np.exp(x, dtype=np.float32)
        w = e / e.sum(axis=1, keepdims=True, dtype=np.float32)

    nl = lab[topi].astype(np.int64)                              # [B, 200]
    flat_base = np.arange(B, dtype=np.int64)[:, None] * NUM_CLASSES
    probas = []
    for k in NB_KNN:
        p = np.bincount((nl[:, :k] + flat_base).ravel(),
                        weights=w[:, :k].astype(np.float64).ravel(),
                        minlength=B * NUM_CLASSES)
        probas.append(p.reshape(B, NUM_CLASSES).astype(np.float32))
    return tuple(probas)
